# revision 25
# baseline (speedup 1.0000x reference)
"""Mamba block kernel for 8 Trainium2 NeuronCores.

Sharding: core c handles batch c//2 and d_inner half c%2 (DL=1024).
x_proj partials are pair-AllReduced ([96,L] f32); out_proj partials are
pair-ReduceScattered ([DM,L] bf16, each core emits half of d_model).

This environment steps instructions at ~30-90us each regardless of size,
so the kernel minimizes INSTRUCTION COUNT:
- all GEMMs in f32r (f32r matmuls emit no separate LDWEIGHTS),
- the 16 scan states per d-tile run as ONE tensor_tensor_scan over the
  flattened (n, t) free dim, with a zero-decay flush column per segment
  that both resets the state and injects the chunk carry,
- the sum over n (y_t = sum_n C h + u D) is a SECOND add-scan over a
  (t, n)-major buffer with a flush slot, so it is 1 instruction too,
- B/C rows are partition-broadcast with a single stride-0 DMA per chunk,
- conv1d is 4 DVE ops per d-tile (scalar_tensor_tensor shift-MACs),
- PSUM epilogues read 4 banks in one instruction,
- weights are pre-arranged on the host so every DMA is 128 contiguous
  per-partition descriptors.
"""
import sys
sys.path.insert(0, "/opt/trn_rl_repo")
import numpy as np
import concourse.bacc as bacc
import concourse.mybir as mybir
from concourse.tile import TileContext
from concourse.bass_utils import run_bass_kernel_spmd

F32 = mybir.dt.float32
F32R = mybir.dt.float32r
BF16 = mybir.dt.bfloat16
OP = mybir.AluOpType
AF = mybir.ActivationFunctionType

B_, L, DM = 4, 2048, 1024       # batch, seqlen, d_model
DI = 2048                        # d_inner (global)
DL = 1024                        # d_inner per core
N = 16                           # d_state
RK = 64                          # dt_rank
KC = 4                           # conv width
TC = 512                         # time chunk (scan + out_proj stage)
NCH = L // TC                    # 4 chunks
NJ = DL // 128                   # 8 d-tiles per core
NK = DM // 128                   # 8 k-tiles over d_model
NM = DM // 128                   # 8 out d_model tiles
TP = TC + 1                      # scan segment length (flush col + TC)
NS = N + 2                       # y-scan slots: flush + 16 n + uD
PAIRS = [[0, 1], [2, 3], [4, 5], [6, 7]]

_CACHED_NC = {}


def _build(reps=1, stages="all"):
    nc = bacc.Bacc(num_devices=8)

    # ---- parameters (per-core shards; host pre-arranged layouts) ----
    hst = nc.declare_dram_parameter("hst", [128, NK * L], F32, isOutput=False)
    wx = nc.declare_dram_parameter("wx", [128, NJ * NK * 128], F32,
                                   isOutput=False)
    wz = nc.declare_dram_parameter("wz", [128, NJ * NK * 128], F32,
                                   isOutput=False)
    wo = nc.declare_dram_parameter("wo", [128, NM * NJ * 128], F32,
                                   isOutput=False)
    wxp = nc.declare_dram_parameter("wxp", [128, NJ * (RK + 2 * N)], F32,
                                    isOutput=False)
    wdt = nc.declare_dram_parameter("wdt", [RK, DL], F32, isOutput=False)
    convw = nc.declare_dram_parameter("convw", [128, NJ * KC], F32,
                                      isOutput=False)
    cbd = nc.declare_dram_parameter("cbd", [128, 3 * NJ], F32, isOutput=False)
    negA = nc.declare_dram_parameter("negA", [128, NJ * N], F32,
                                     isOutput=False)
    oslab = nc.declare_dram_parameter("oslab", [DM // 2, L], BF16,
                                      isOutput=True)

    with TileContext(nc) as tc:
        with tc.tile_pool(name="const", bufs=1) as cp:
            convw_t = cp.tile([128, NJ, KC], F32, tag="convw", name="convw_t")
            nc.sync.dma_start(
                out=convw_t[:, :, :],
                in_=convw[:, :].rearrange("p (j k) -> p j k", j=NJ))
            cbd_t = cp.tile([128, 3 * NJ], F32, tag="cbd", name="cbd_t")
            nc.sync.dma_start(out=cbd_t[:, :], in_=cbd[:, :])
            negA_t = cp.tile([128, NJ, N], F32, tag="negA", name="negA_t")
            nc.sync.dma_start(
                out=negA_t[:, :, :],
                in_=negA[:, :].rearrange("p (j n) -> p j n", j=NJ))
            wxp_t = cp.tile([128, NJ, RK + 2 * N], F32R, tag="wxp",
                            name="wxp_t")
            nc.sync.dma_start(
                out=wxp_t[:, :, :],
                in_=wxp[:, :].rearrange("p (j w) -> p j w",
                                        j=NJ).bitcast(F32R))
            wdt_t = cp.tile([RK, DL], F32R, tag="wdt", name="wdt_t")
            nc.sync.dma_start(out=wdt_t[:, :], in_=wdt[:, :].bitcast(F32R))
            ones_t = cp.tile([128, TC, NS], BF16, tag="ones", name="ones_t")
            nc.vector.memset(ones_t[:, :, :], 1.0)
            nc.vector.memset(ones_t[:, :, 0], 0.0)
            carry = cp.tile([128, NJ, N], F32, tag="carry", name="carry_t")
            ubf = cp.tile([128, NJ, L], BF16, tag="ubf", name="ubf_t")

            for rep in range(reps):
                xdbl_in = nc.dram_tensor(f"xdbl_in{rep}", [RK + 2 * N, L],
                                         BF16)
                xdbl_out = nc.dram_tensor(f"xdbl_out{rep}", [RK + 2 * N, L],
                                          BF16)
                dg_dram = nc.dram_tensor(f"dg_dram{rep}",
                                         [NCH, 128, 2, NJ, TC], BF16)
                oc_in = nc.dram_tensor(f"oc_in{rep}", [DM, L], BF16)
                oc_out = nc.dram_tensor(f"oc_out{rep}", [DM // 2, L], BF16)

                with tc.tile_pool(name=f"hsp{rep}", bufs=1) as hp:
                    hsT = hp.tile([128, NK, L], F32R, tag="hsT", name="hsT")
                    nc.sync.dma_start(
                        out=hsT[:, :, :],
                        in_=hst[:, :].rearrange("p (k t) -> p k t",
                                                k=NK).bitcast(F32R))
                    if stages in ("all", "gemm"):
                        _emit_in_x(nc, tc, rep, wx, hsT, convw_t, cbd_t,
                                   wxp_t, ubf, xdbl_in)
                        nc.gpsimd.collective_compute(
                            "AllReduce", OP.add, replica_groups=PAIRS,
                            ins=[xdbl_in[:, :]], outs=[xdbl_out[:, :]])
                        _emit_in_z(nc, tc, rep, wz, hsT, dg_dram)
                    else:
                        nc.sync.dma_start(out=xdbl_out[:, :],
                                          in_=xdbl_in[:, :])
                        with tc.tile_pool(name=f"fz{rep}", bufs=1) as fz:
                            zzz = fz.tile([128, NJ, L], BF16, tag="zzz",
                                          name="zzz")
                            nc.vector.memset(zzz[:, :, :], 0.5)
                            nc.vector.memset(ubf[:, :, :], 0.5)
                            nc.sync.dma_start(
                                out=dg_dram[:, :, 1, :, :].transpose(
                                    [1, 2, 0, 3]),
                                in_=zzz[:, :, :].rearrange(
                                    "p j (c t) -> p j c t", c=NCH))

                # dt path hoisted: all Exp first, then all Ln (no act-table
                # thrash); dt spilled to dram in bf16 chunk-blocked layout
                with (
                    tc.tile_pool(name=f"dts{rep}", bufs=1) as dp,
                    tc.tile_pool(name=f"pdt{rep}", bufs=2, space="PSUM")
                    as pdt,
                ):
                    drf = dp.tile([RK, L], BF16, tag="drf", name="drf")
                    nc.sync.dma_start(out=drf[:, :], in_=xdbl_out[0:RK, :])
                    dtraw = dp.tile([RK, L], F32R, tag="dtraw", name="dtraw")
                    nc.vector.tensor_scalar(dtraw[:, :], drf[:, :], 0.0, 1.0,
                                            op0=OP.max, op1=OP.min)
                    spe_all = dp.tile([128, NJ, L], F32, tag="spe",
                                      name="spe_all")
                    for j in range(NJ):
                        dps = pdt.tile([128, 4, TC], F32, tag="dps",
                                       name="dps")
                        for q in range(4):
                            nc.tensor.matmul(
                                dps[:, q, :],
                                wdt_t[:, j * 128:(j + 1) * 128],
                                dtraw[:, q * TC:(q + 1) * TC],
                                start=True, stop=True)
                        nc.scalar.activation(
                            spe_all[:, j, :],
                            dps[:, :, :].rearrange("p q t -> p (q t)"),
                            AF.Exp, bias=cbd_t[:, NJ + j:NJ + j + 1])
                    dtt = dp.tile([128, L], BF16, tag="dtt", name="dtt")
                    for j in range(NJ):
                        nc.scalar.activation(dtt[:, :], spe_all[:, j, :],
                                             AF.Ln, bias=1.0)
                        nc.sync.dma_start(
                            out=dg_dram[:, :, 0, j, :].transpose([1, 0, 2]),
                            in_=dtt[:, :].rearrange("p (c t) -> p c t",
                                                    c=NCH))

                if stages in ("all", "scan"):
                    _emit_scan(nc, tc, rep, cbd_t, negA_t, wo, ones_t,
                               carry, ubf, dg_dram, xdbl_out, oc_in)
                    nc.gpsimd.collective_compute(
                        "ReduceScatter", OP.add, replica_groups=PAIRS,
                        ins=[oc_in[:, :]], outs=[oc_out[:, :]])
                    nc.gpsimd.dma_start(out=oslab[:, :], in_=oc_out[:, :])
                else:
                    nc.gpsimd.dma_start(out=oslab[:, :],
                                        in_=oc_out[:, :])

    nc.finalize()
    return nc


def _emit_in_x(nc, tc, rep, wx, hsT, convw_t, cbd_t, wxp_t, ubf, xdbl_in):
    """in_proj x-side + conv + silu + clip + x_proj partials."""
    with (
        tc.tile_pool(name=f"wxp{rep}", bufs=1) as wp,
        tc.tile_pool(name=f"xwork{rep}", bufs=1) as xw,
        tc.tile_pool(name=f"psx{rep}", bufs=1, space="PSUM") as psx,
        tc.tile_pool(name=f"psg{rep}", bufs=1, space="PSUM") as psg,
    ):
        wxt = wp.tile([128, NJ, NK, 128], F32R, tag="wx", name="wxt")
        nc.sync.dma_start(
            out=wxt[:, :, :, :],
            in_=wx[:, :].rearrange("p (j k q) -> p j k q",
                                   j=NJ, k=NK).bitcast(F32R))
        psxs = psx.tile([RK + 2 * N, 4, TC], F32, tag="psx", name="psxs")
        xps = psg.tile([128, 4, TC], F32, tag="xps", name="xps")
        xcj = xw.tile([128, KC - 1 + L], BF16, tag="xcj", name="xcj")
        nc.vector.memset(xcj[:, 0:KC - 1], 0.0)
        cv = xw.tile([128, L], F32, tag="cv", name="cv")
        tmp = xw.tile([128, L], F32, tag="tmp", name="tmp")
        uf = xw.tile([128, L], F32R, tag="uf", name="uf")
        for j in range(NJ):
            for k in range(NK):
                for q in range(4):
                    nc.tensor.matmul(
                        xps[:, q, :], wxt[:, j, k, :],
                        hsT[:, k, q * TC:(q + 1) * TC],
                        start=(k == 0), stop=(k == NK - 1))
            nc.vector.tensor_scalar(
                xcj[:, KC - 1:],
                xps[:, :, :].rearrange("p q t -> p (q t)"),
                0.0, 1.0, op0=OP.max, op1=OP.min)
            nc.vector.tensor_tensor(
                out=cv[:, :], in0=xcj[:, 0:L],
                in1=convw_t[:, j, 0:1].broadcast_to([128, L]), op=OP.mult)
            for k in range(1, KC):
                # cv = (x_shifted * w_k) + cv  (fused 3-operand DVE op)
                nc.vector.scalar_tensor_tensor(
                    out=cv[:, :], in0=xcj[:, k:k + L],
                    scalar=convw_t[:, j, k:k + 1], in1=cv[:, :],
                    op0=OP.mult, op1=OP.add)
            nc.scalar.activation(tmp[:, :], cv[:, :], AF.Silu,
                                 bias=cbd_t[:, j:j + 1])
            nc.vector.tensor_scalar(uf[:, :], tmp[:, :], 0.0, 1.0,
                                    op0=OP.max, op1=OP.min)
            nc.vector.tensor_copy(ubf[:, j, :], uf[:, :].bitcast(F32))
            for q in range(4):
                nc.tensor.matmul(
                    psxs[:, q, :], wxp_t[:, j, :],
                    uf[:, q * TC:(q + 1) * TC],
                    start=(j == 0), stop=(j == NJ - 1))
        xdbl = xw.tile([RK + 2 * N, L], BF16, tag="xdbl", name="xdbl")
        nc.scalar.copy(xdbl[:, :],
                       psxs[:, :, :].rearrange("p q t -> p (q t)"))
        nc.sync.dma_start(out=xdbl_in[:, :], in_=xdbl[:, :])


def _emit_in_z(nc, tc, rep, wz, hsT, dg_dram):
    """in_proj z-side + clip + silu -> gate (overlaps the AllReduce)."""
    with (
        tc.tile_pool(name=f"wzp{rep}", bufs=1) as wp,
        tc.tile_pool(name=f"zwork{rep}", bufs=1) as zw,
        tc.tile_pool(name=f"psz{rep}", bufs=2, space="PSUM") as psz,
    ):
        wzt = wp.tile([128, NJ, NK, 128], F32R, tag="wz", name="wzt")
        nc.sync.dma_start(
            out=wzt[:, :, :, :],
            in_=wz[:, :].rearrange("p (j k q) -> p j k q",
                                   j=NJ, k=NK).bitcast(F32R))
        zb = zw.tile([128, L], BF16, tag="zb", name="zb")
        gt = zw.tile([128, L], BF16, tag="gt", name="gt")
        for j in range(NJ):
            zps = psz.tile([128, 4, TC], F32, tag="zps", name="zps")
            for k in range(NK):
                for q in range(4):
                    nc.tensor.matmul(
                        zps[:, q, :], wzt[:, j, k, :],
                        hsT[:, k, q * TC:(q + 1) * TC],
                        start=(k == 0), stop=(k == NK - 1))
            nc.vector.tensor_scalar(
                zb[:, :], zps[:, :, :].rearrange("p q t -> p (q t)"),
                0.0, 1.0, op0=OP.max, op1=OP.min)
            nc.scalar.activation(gt[:, :], zb[:, :], AF.Silu)
            nc.sync.dma_start(
                out=dg_dram[:, :, 1, j, :].transpose([1, 0, 2]),
                in_=gt[:, :].rearrange("p (c t) -> p c t", c=NCH))


def _emit_scan(nc, tc, rep, cbd_t, negA_t, wo, ones_t, carry,
               ubf, dg_dram, xdbl_out, oc_in):
    """selective scan + gate + out_proj partials, per chunk."""
    with (
        tc.tile_pool(name=f"chk{rep}", bufs=1) as ck,
        tc.tile_pool(name=f"jw{rep}", bufs=1) as jw,
        tc.tile_pool(name=f"sm{rep}", bufs=1) as sm,
        tc.tile_pool(name=f"pop{rep}", bufs=4, space="PSUM") as pop,
    ):
        an = jw.tile([128, N, TP], F32, tag="an", name="an")
        bt = jw.tile([128, N, TP], BF16, tag="bt", name="bt")
        ch = jw.tile([128, TC, NS], BF16, tag="ch", name="ch")
        yg = jw.tile([128, NJ, TC], F32R, tag="yg", name="yg")
        yt = sm.tile([128, TC], BF16, tag="yt", name="yt")
        osb = sm.tile([128, NM, TC], BF16, tag="osb", name="osb")
        nc.vector.memset(ch[:, :, 0], 0.0)
        nc.vector.memset(an[:, :, 0], 0.0)
        for c in range(NCH):
            csl = slice(c * TC, (c + 1) * TC)
            bcall = ck.tile([128, 2 * N, TC], BF16, tag="bcall", name="bcall")
            nc.sync.dma_start(
                out=bcall[:, :, :],
                in_=xdbl_out[RK:RK + 2 * N, csl][None, :, :].broadcast_to(
                    [128, 2 * N, TC]))
            dtg_c = ck.tile([128, 2, NJ, TC], BF16, tag="dtg_c",
                            name="dtg_c")
            nc.sync.dma_start(out=dtg_c[:, :, :, :],
                              in_=dg_dram[c, :, :, :, :])
            dt_c = dtg_c[:, 0]
            g_c = dtg_c[:, 1]
            # dt*u for all 8 d-tiles in one op (shares the idle wom slot)
            dtu_c = sm.tile([128, NJ, TC], BF16, tag="wom", name="dtu_c")
            nc.vector.tensor_tensor(out=dtu_c[:, :, :], in0=dt_c[:, :, :],
                                    in1=ubf[:, :, csl], op=OP.mult)
            for j in range(NJ):
                # an[:, n, 1+t] = exp(negA[n] * dt[t]); col 0 stays 0 (flush)
                nc.vector.tensor_tensor(
                    out=an[:, :, 1:],
                    in0=dt_c[:, j, None, :].broadcast_to([128, N, TC]),
                    in1=negA_t[:, j, :, None].broadcast_to([128, N, TC]),
                    op=OP.mult)
                nc.scalar.activation(an[:, :, 1:], an[:, :, 1:], AF.Exp)
                # carry inject into flush col of bt
                if c == 0:
                    nc.vector.memset(bt[:, :, 0], 0.0)
                else:
                    nc.vector.tensor_copy(bt[:, :, 0], carry[:, j, :])
                nc.vector.tensor_tensor(
                    out=bt[:, :, 1:],
                    in0=dtu_c[:, j, None, :].broadcast_to([128, N, TC]),
                    in1=bcall[:, 0:N, :], op=OP.mult)
                # fused 16-state scan; output overwrites bt (ht := bt)
                nc.vector.tensor_tensor_scan(
                    bt[:, :, :].rearrange("p n t -> p (n t)"),
                    an[:, :, :].rearrange("p n t -> p (n t)"),
                    bt[:, :, :].rearrange("p n t -> p (n t)"), 0.0,
                    op0=OP.mult, op1=OP.add)
                if c < NCH - 1:
                    nc.vector.tensor_copy(carry[:, j, :], bt[:, :, TP - 1])
                # ch[t, 1+n] = h * C ; ch[t, 17] = u * D ; col 0 stays 0
                nc.vector.tensor_tensor(
                    out=ch[:, :, 1:N + 1].transpose([0, 2, 1]),
                    in0=bt[:, :, 1:], in1=bcall[:, N:2 * N, :], op=OP.mult)
                nc.vector.tensor_tensor(
                    out=ch[:, :, N + 1], in0=ubf[:, j, csl],
                    in1=cbd_t[:, 2 * NJ + j:2 * NJ + j + 1].broadcast_to(
                        [128, TC]), op=OP.mult)
                # y-scan: sum over n slots (in-place over ch)
                nc.vector.tensor_tensor_scan(
                    ch[:, :, :].rearrange("p t n -> p (t n)"),
                    ones_t[:, :, :].rearrange("p t n -> p (t n)"),
                    ch[:, :, :].rearrange("p t n -> p (t n)"), 0.0,
                    op0=OP.mult, op1=OP.add)
                nc.vector.tensor_scalar(yt[:, :], ch[:, :, N + 1],
                                        0.0, 1.0, op0=OP.max, op1=OP.min)
                nc.vector.tensor_tensor(out=yg[:, j, :],
                                        in0=yt[:, :], in1=g_c[:, j, :],
                                        op=OP.mult)
            # out_proj for this chunk (weights streamed per m-pair)
            for mq in range(NM // 2):
                wom = sm.tile([128, 2, NJ, 128], F32R, tag="wom", name="wom")
                nc.sync.dma_start(
                    out=wom[:, :, :, :],
                    in_=wo[:, mq * 2 * NJ * 128:(mq + 1) * 2 * NJ * 128]
                    .rearrange("p (m j q) -> p m j q", m=2,
                               j=NJ).bitcast(F32R))
                ops = pop.tile([128, 2, TC], F32, tag="ops", name="ops")
                for mm in range(2):
                    for j in range(NJ):
                        nc.tensor.matmul(ops[:, mm, :], wom[:, mm, j, :],
                                         yg[:, j, :], start=(j == 0),
                                         stop=(j == NJ - 1))
                nc.scalar.copy(
                    osb[:, 2 * mq:2 * mq + 2, :],
                    ops[:, :, :])
            nc.sync.dma_start(
                out=oc_in[:, csl].rearrange("(m p) t -> p m t", m=NM),
                in_=osb[:, :, :])


def _shard(inputs):
    hs = np.asarray(inputs["hidden_states"], np.float32)
    W_in = np.asarray(inputs["W_in"], np.float32)
    conv_w = np.asarray(inputs["conv_w"], np.float32)
    conv_b = np.asarray(inputs["conv_b"], np.float32)
    W_x = np.asarray(inputs["W_x"], np.float32)
    W_dt = np.asarray(inputs["W_dt"], np.float32)
    b_dt = np.asarray(inputs["b_dt"], np.float32)
    W_out = np.asarray(inputs["W_out"], np.float32)
    A_log = np.asarray(inputs["A_log"], np.float32)
    D = np.asarray(inputs["D"], np.float32)

    in_maps = []
    for c in range(8):
        b, dh = c // 2, c % 2
        dsl = slice(dh * DL, (dh + 1) * DL)
        convw_l = np.ascontiguousarray(
            conv_w[dsl, 0, :].reshape(NJ, 128, KC).transpose(1, 0, 2)
            .reshape(128, NJ * KC))
        cbd_l = np.concatenate([
            conv_b[dsl].reshape(NJ, 128).T,
            b_dt[dsl].reshape(NJ, 128).T,
            D[dsl].reshape(NJ, 128).T], axis=1)
        negA_l = np.ascontiguousarray(
            (-np.exp(A_log[dsl])).reshape(NJ, 128, N).transpose(1, 0, 2)
            .reshape(128, NJ * N))
        # pre-arranged layouts: one contiguous block per partition
        hst_l = np.ascontiguousarray(
            hs[b].T.reshape(NK, 128, L).transpose(1, 0, 2).reshape(128, -1))
        wxT = W_in[dsl].T          # [DM, DL]
        wx_l = np.ascontiguousarray(
            wxT.reshape(NK, 128, NJ, 128).transpose(1, 2, 0, 3)
            .reshape(128, -1))
        wzT = W_in[DI + dh * DL: DI + (dh + 1) * DL].T
        wz_l = np.ascontiguousarray(
            wzT.reshape(NK, 128, NJ, 128).transpose(1, 2, 0, 3)
            .reshape(128, -1))
        woT = W_out[:, dsl].T      # [DL, DM]
        wo_l = np.ascontiguousarray(
            woT.reshape(NJ, 128, NM, 128).transpose(1, 2, 0, 3)
            .reshape(128, -1))
        wxpT = W_x[:, dsl].T       # [DL, 96]
        wxp_l = np.ascontiguousarray(
            wxpT.reshape(NJ, 128, RK + 2 * N).transpose(1, 0, 2)
            .reshape(128, -1))
        m = {
            "hst": hst_l,
            "wx": wx_l,
            "wz": wz_l,
            "wo": wo_l,
            "wxp": wxp_l,
            "wdt": np.ascontiguousarray(W_dt[dsl].T),
            "convw": convw_l,
            "cbd": np.ascontiguousarray(cbd_l),
            "negA": negA_l,
        }
        in_maps.append(m)
    return in_maps


def kernel(**inputs):
    if 1 not in _CACHED_NC:
        _CACHED_NC[1] = _build(1)
    nc = _CACHED_NC[1]
    in_maps = _shard(inputs)
    res = run_bass_kernel_spmd(nc, in_maps, core_ids=list(range(8)))
    out = np.empty((B_, L, DM), np.float32)
    for b in range(B_):
        s0 = np.asarray(res.results[2 * b]["oslab"], dtype=np.float32)
        s1 = np.asarray(res.results[2 * b + 1]["oslab"], dtype=np.float32)
        out[b] = np.concatenate([s0, s1], axis=0).T
    return out
